# revision 8
# baseline (speedup 1.0000x reference)
# MBartAttention single-step decode with KV-cache scatter, Trainium2 Bass/Tile.
#
# Sharding: data-parallel over batch. 16 batches / 8 cores = 2 batches per
# core; each core runs an identical program over its batch shard (no
# collectives). Host pre-transposes the (tiny) weight matrices so the device
# never has to transpose DRAM-resident weights.
#
# Per-core device work:
#   q/k/v = hs @ W.T + b       (PE; contraction over D on partitions; biases
#                               via an appended ones-row / bias-row)
#   kb/vb = scatter(step)      (head [0,step) streamed through SBUF and copied
#                               out; tail (step, S) copied DRAM->DRAM; row
#                               `step` written from the fresh k/v projection)
#   scores = K . q             (PE; K tiles transposed on PE via identity, then
#                               used as stationary so scores land with s on
#                               partitions -- the same layout V is consumed in)
#   softmax                    (ACT exp with per-(b,h) accumulate; partition
#                               reduction + reciprocal broadcast via tiny PE
#                               ones-matmuls; no max subtraction -- scores are
#                               O(1) here so exp cannot overflow)
#   ctx = sum_s p[s] V[s]      (PE; V consumed in natural [s, hd] layout from
#                               the same SBUF-resident copy used for copy-out)
#   out = ctx @ Wo.T + bo      (PE)
#
# The kernel is memory-bound: per core it must read K,V heads+tails+weights
# (~80 MiB) and write the updated caches (~64 MiB).

import numpy as np

import concourse.bass as bass  # noqa: F401
import concourse.mybir as mybir
import concourse.tile as tile
from concourse import bacc
from concourse.bass_utils import run_bass_kernel_spmd
from concourse.masks import make_identity

FP32 = mybir.dt.float32
AX = mybir.AxisListType
ALU = mybir.AluOpType
EXP = mybir.ActivationFunctionType.Exp

B, H, S, HD = 16, 16, 4096, 64
D = H * HD              # 1024
NCORES = 8
BL = B // NCORES        # 2 batches per core
NBH = BL * H            # 32 (batch, head) pairs per core


def _emit(tc, step, hsT, wqkvT, woT, bo_c, kbuf, vbuf, out_d, kb_d, vb_d, stage=99):
    nc = tc.nc
    HEAD = step             # cached positions attended: 0..step-1 (+ new at step)
    assert HEAD % 512 == 0
    FPP = HEAD // 128       # head positions per SBUF partition (j slots)
    JP1 = FPP + 1           # + slot for the freshly projected position `step`
    NCH = HEAD // 512       # 512-column K-transpose chunks
    DT = D // 128           # din tiles
    DTP = DT + 1            # + bias (ones-row) tile

    with (
        tc.tile_pool(name="const", bufs=1) as cpool,
        tc.tile_pool(name="vres", bufs=1) as vpool,
        tc.tile_pool(name="small", bufs=1) as spool,
    ):
        ident = cpool.tile([128, 128], FP32, name="ident")
        make_identity(nc, ident[:])
        ones1 = cpool.tile([1, 1], FP32, name="ones1")
        nc.vector.memset(ones1[:], 1.0)
        ones_col = cpool.tile([128, 1], FP32, name="ones_col")
        nc.vector.memset(ones_col[:], 1.0)
        ones_row = cpool.tile([1, 128], FP32, name="ones_row")
        nc.vector.memset(ones_row[:], 1.0)
        bo_sb = cpool.tile([1, D], FP32, name="bo_sb")
        nc.sync.dma_start(bo_sb[:], bo_c[:, :])

        hsT_sb = spool.tile([128, DTP, BL], FP32, name="hsT_sb")
        nc.sync.dma_start(hsT_sb[:], hsT[:, :, :])

        # V head cache, resident: [s%128, s//128, bh, hd]; j slot FPP holds the
        # new position `step` in partition 0 (other partitions zeroed).
        v_sb = vpool.tile([128, JP1, NBH, HD], FP32, name="v_sb")
        nc.vector.memset(v_sb[:, FPP, :, :], 0.0)

        # exp(scores), layout [s%128, h, j, b]; j slot FPP = position `step`
        # (partition 0; rest zeroed).
        probs = spool.tile([128, H, JP1, BL], FP32, name="probs")
        nc.vector.memset(probs[:, :, FPP, :], 0.0)
        acc_sb = spool.tile([128, NBH], FP32, name="acc_sb")

        qp_sb = spool.tile([BL, D], FP32, name="qp_sb")
        knp_sb = spool.tile([BL, D], FP32, name="knp_sb")
        vnp_sb = spool.tile([BL, D], FP32, name="vnp_sb")
        qT_sb = spool.tile([64, H, BL], FP32, name="qT_sb")
        knT_sb = spool.tile([64, H, BL], FP32, name="knT_sb")
        rd_row = spool.tile([1, NBH], FP32, name="rd_row")
        ctx_sb = spool.tile([64, NBH], FP32, name="ctx_sb")
        out_sb = spool.tile([1, BL, D], FP32, name="out_sb")

        # ---------------- q/k/v projections ----------------
        with (
            tc.tile_pool(name="wqkv", bufs=2) as wpool,
            tc.tile_pool(name="proj_ps", bufs=1, space="PSUM") as proj_psum,
            tc.tile_pool(name="tr_ps", bufs=1, space="PSUM") as tr_psum,
        ):
            pps = [proj_psum.tile([BL, 512], FP32, name=f"pp{c}") for c in range(6)]
            for d in range(DTP):
                w_sb = wpool.tile([128, 3 * D], FP32, name="w_sb", tag="w")
                nc.sync.dma_start(w_sb[:], wqkvT[128 * d:128 * (d + 1), :])
                for c in range(6):
                    nc.tensor.matmul(
                        pps[c][:], hsT_sb[:, d, :], w_sb[:, 512 * c:512 * (c + 1)],
                        start=(d == 0), stop=(d == DTP - 1),
                    )
            for c in range(2):
                sl = slice(512 * c, 512 * (c + 1))
                nc.scalar.copy(qp_sb[:, sl], pps[c][:])
                nc.scalar.copy(knp_sb[:, sl], pps[2 + c][:])
                nc.scalar.copy(vnp_sb[:, sl], pps[4 + c][:])

            # scatter the new K/V row into the caches at position `step`
            nc.sync.dma_start(
                kb_d[:, :, step, :], knp_sb[:].rearrange("b (h e) -> b h e", h=H))
            nc.sync.dma_start(
                vb_d[:, :, step, :], vnp_sb[:].rearrange("b (h e) -> b h e", h=H))
            # and into the resident V buffer (slot FPP, partition 0)
            for b in range(BL):
                nc.sync.dma_start(
                    v_sb[0:1, FPP, H * b:H * (b + 1), :],
                    vnp_sb[b:b + 1, :].rearrange("p (h e) -> p h e", h=H))

            # q, k_new transposed to [hd, h, b] for PE score matmuls
            qT_ps = tr_psum.tile([64, H * BL], FP32, name="qT_ps")
            knT_ps = tr_psum.tile([64, H * BL], FP32, name="knT_ps")
            for h in range(H):
                sl = slice(64 * h, 64 * (h + 1))
                nc.tensor.transpose(
                    qT_ps[:, h * BL:(h + 1) * BL], qp_sb[:, sl], ident[:BL, :BL])
                nc.tensor.transpose(
                    knT_ps[:, h * BL:(h + 1) * BL], knp_sb[:, sl], ident[:BL, :BL])
            nc.vector.tensor_copy(qT_sb[:].rearrange("p h b -> p (h b)"), qT_ps[:])
            nc.vector.tensor_copy(knT_sb[:].rearrange("p h b -> p (h b)"), knT_ps[:])

        # ---------------- K/V stream + scores ----------------
        with (
            tc.tile_pool(name="kst", bufs=2) as kpool,
            tc.tile_pool(name="ktp", bufs=2, space="PSUM") as kt_psum,
            tc.tile_pool(name="kts", bufs=2) as kts_pool,
            tc.tile_pool(name="sc_ps", bufs=1, space="PSUM") as sc_psum,
        ):
            # scores: [s%128, (h, j, b)] -- one PSUM bank; plus a [1, (h, b)]
            # row for the position-`step` scores.
            scores = sc_psum.tile([128, H, FPP, BL], FP32, name="scores")
            scn = sc_psum.tile([1, H * BL], FP32, name="scn")
            for b in range(BL):
                for h in range(H):
                    bh = b * H + h
                    k_sb = kpool.tile([128, FPP, HD], FP32, name="k_sb", tag="k")
                    nc.sync.dma_start(
                        k_sb[:],
                        kbuf[b, h, 0:HEAD, :].rearrange("(p f) e -> p f e", p=128))
                    nc.sync.dma_start(
                        kb_d[b, h, 0:HEAD, :].rearrange("(p f) e -> p f e", p=128),
                        k_sb[:])
                    nc.sync.dma_start(
                        kb_d[b, h, step + 1:S, :], kbuf[b, h, step + 1:S, :])
                    for c in range(NCH if stage >= 2 else 0):
                        ktp = kt_psum.tile([64, 512], FP32, name="ktp", tag="ktp")
                        for j4 in range(4):
                            nc.tensor.transpose(
                                ktp[:, 128 * j4:128 * (j4 + 1)],
                                k_sb[:, 4 * c + j4, :], ident[:])
                        kts = kts_pool.tile([64, 512], FP32, name="kts", tag="kts")
                        if (bh + c) % 2 == 0:
                            nc.scalar.copy(kts[:], ktp[:])
                        else:
                            nc.vector.tensor_copy(kts[:], ktp[:])
                        for j4 in range(4):
                            j = 4 * c + j4
                            nc.tensor.matmul(
                                scores[:, h, j, b:b + 1],
                                kts[:, 128 * j4:128 * (j4 + 1)],
                                qT_sb[:, h, b:b + 1], start=True, stop=True)
                    if stage >= 2:
                      nc.tensor.matmul(
                        scn[0:1, h * BL + b:h * BL + b + 1],
                        knT_sb[:, h, b:b + 1], qT_sb[:, h, b:b + 1],
                        start=True, stop=True)
                    # V: load head into resident buffer, copy out, tail D2D
                    nc.sync.dma_start(
                        v_sb[:, 0:FPP, bh, :],
                        vbuf[b, h, 0:HEAD, :].rearrange("(p f) e -> p f e", p=128))
                    nc.sync.dma_start(
                        vb_d[b, h, 0:HEAD, :].rearrange("(p f) e -> p f e", p=128),
                        v_sb[:, 0:FPP, bh, :])
                    nc.sync.dma_start(
                        vb_d[b, h, step + 1:S, :], vbuf[b, h, step + 1:S, :])

            if stage < 3:
                return
            # ---------------- softmax ----------------
            # exp; per-(b,h) partial sums land in acc_sb columns
            for b in range(BL):
                for h in range(H):
                    bh = b * H + h
                    nc.scalar.activation(
                        probs[:, h, 0:FPP, b], scores[:, h, :, b], EXP,
                        accum_out=acc_sb[:, bh:bh + 1])
            nc.scalar.activation(probs[0:1, :, FPP, :], scn[:, :], EXP)

            with tc.tile_pool(name="den_ps", bufs=1, space="PSUM") as den_psum:
                den_row = den_psum.tile([1, NBH], FP32, name="den_row")
                rbc = den_psum.tile([128, NBH], FP32, name="rbc")
                # den[b*H+h] = sum over partitions of acc + the `step` term
                nc.tensor.matmul(
                    den_row[:], ones_col[:], acc_sb[:], start=True, stop=False)
                nc.tensor.matmul(
                    den_row[:], ones1[:],
                    probs[0:1, :, FPP, :].rearrange("p h b -> p b h"),
                    start=False, stop=True)
                nc.vector.reciprocal(rd_row[:], den_row[:])
                # broadcast 1/den down all partitions, then scale probs
                nc.tensor.matmul(
                    rbc[:], ones_row[:], rd_row[:], start=True, stop=True)
                nc.vector.tensor_mul(
                    probs[:],
                    probs[:],
                    rbc[:].rearrange("p (b h one) -> p h one b", b=BL, one=1)
                        .broadcast_to([128, H, JP1, BL]))

        if stage < 4:
            return
        # ---------------- ctx = p . V ----------------
        with tc.tile_pool(name="ctx_ps", bufs=1, space="PSUM") as ctx_psum:
            ctxp = ctx_psum.tile([64, NBH], FP32, name="ctxp")
            for b in range(BL):
                for h in range(H):
                    bh = b * H + h
                    for m in range(JP1):
                        nc.tensor.matmul(
                            ctxp[:, bh:bh + 1], v_sb[:, m, bh, :],
                            probs[:, h, m, b:b + 1],
                            start=(m == 0), stop=(m == JP1 - 1))
            nc.vector.tensor_copy(ctx_sb[:], ctxp[:])

        if stage < 5:
            return
        # ---------------- out = ctx @ Wo.T + bo ----------------
        with (
            tc.tile_pool(name="wo", bufs=1) as wo_pool,
            tc.tile_pool(name="op_ps", bufs=1, space="PSUM") as op_psum,
        ):
            opst = [[op_psum.tile([1, 512], FP32, name=f"ops{c2}{b}")
                     for b in range(BL)] for c2 in range(2)]
            for c2 in range(2):
                for b in range(BL):
                    nc.tensor.matmul(
                        opst[c2][b][:], ones1[:],
                        bo_sb[0:1, 512 * c2:512 * (c2 + 1)],
                        start=True, stop=False)
            HP = H // 2
            for pc in range(2):
                # Wo.T rows for heads [8*pc, 8*pc+8), laid out [hd, h, n]
                wop = wo_pool.tile([64, HP, D], FP32, name="wop", tag="wop")
                nc.sync.dma_start(
                    wop[:],
                    woT[64 * HP * pc:64 * HP * (pc + 1), :]
                        .rearrange("(h p) n -> p h n", p=64))
                for c2 in range(2):
                    for b in range(BL):
                        for hh in range(HP):
                            h = HP * pc + hh
                            nc.tensor.matmul(
                                opst[c2][b][:], ctx_sb[:, b * H + h:b * H + h + 1],
                                wop[:, hh, 512 * c2:512 * (c2 + 1)],
                                start=False, stop=(h == H - 1))
            for c2 in range(2):
                for b in range(BL):
                    nc.scalar.copy(
                        out_sb[0:1, b, 512 * c2:512 * (c2 + 1)], opst[c2][b][:])
            for b in range(BL):
                nc.sync.dma_start(out_d[b:b + 1, :], out_sb[0:1, b, :])


def build(step, stage=99):
    nc = bacc.Bacc("TRN2", target_bir_lowering=False, debug=False)
    hsT = nc.dram_tensor("hsT", [128, D // 128 + 1, BL], FP32, kind="ExternalInput")
    wqkvT = nc.dram_tensor("wqkvT", [D + 128, 3 * D], FP32, kind="ExternalInput")
    woT = nc.dram_tensor("woT", [D, D], FP32, kind="ExternalInput")
    bo_c = nc.dram_tensor("bo_c", [1, D], FP32, kind="ExternalInput")
    kbuf = nc.dram_tensor("kbuf", [BL, H, S, HD], FP32, kind="ExternalInput")
    vbuf = nc.dram_tensor("vbuf", [BL, H, S, HD], FP32, kind="ExternalInput")
    out_d = nc.dram_tensor("out", [BL, D], FP32, kind="ExternalOutput")
    kb_d = nc.dram_tensor("kb", [BL, H, S, HD], FP32, kind="ExternalOutput")
    vb_d = nc.dram_tensor("vb", [BL, H, S, HD], FP32, kind="ExternalOutput")
    with tile.TileContext(nc) as tc:
        _emit(tc, step, hsT, wqkvT, woT, bo_c, kbuf, vbuf, out_d, kb_d, vb_d, stage=stage)
    nc.compile()
    return nc


def host_inputs(hidden_states, Wq, bq, Wk, bk, Wv, bv, Wo, bo,
                key_buffer, value_buffer):
    """Build the per-core input maps (host-side layout prep only)."""
    hs = np.asarray(hidden_states, dtype=np.float32)
    scale = np.float32(HD) ** np.float32(-0.5)
    wq = np.asarray(Wq, np.float32) * scale
    bqs = np.asarray(bq, np.float32) * scale
    wqkvT = np.zeros((D + 128, 3 * D), np.float32)
    wqkvT[:D, 0 * D:1 * D] = wq.T
    wqkvT[:D, 1 * D:2 * D] = np.asarray(Wk, np.float32).T
    wqkvT[:D, 2 * D:3 * D] = np.asarray(Wv, np.float32).T
    wqkvT[D, 0 * D:1 * D] = bqs
    wqkvT[D, 1 * D:2 * D] = np.asarray(bk, np.float32)
    wqkvT[D, 2 * D:3 * D] = np.asarray(bv, np.float32)
    woT = np.ascontiguousarray(np.asarray(Wo, np.float32).T)
    bo_c = np.asarray(bo, np.float32).reshape(1, D)

    kb = np.asarray(key_buffer, np.float32)
    vb = np.asarray(value_buffer, np.float32)

    in_maps = []
    for c in range(NCORES):
        b0 = c * BL
        hsc = hs[b0:b0 + BL, 0, :]                       # [BL, D]
        hsT = np.zeros((128, D // 128 + 1, BL), np.float32)
        hsT[:, :D // 128, :] = hsc.T.reshape(D // 128, 128, BL).transpose(1, 0, 2)
        hsT[0, D // 128, :] = 1.0                        # ones row -> bias row
        in_maps.append({
            "hsT": hsT,
            "wqkvT": wqkvT,
            "woT": woT,
            "bo_c": bo_c,
            "kbuf": np.ascontiguousarray(kb[b0:b0 + BL]),
            "vbuf": np.ascontiguousarray(vb[b0:b0 + BL]),
        })
    return in_maps


_build_cache = {}


def kernel(hidden_states, key_buffer, value_buffer, Wq, bq, Wk, bk, Wv, bv,
           Wo, bo, step, _run_opts=None):
    step = int(step)
    if step not in _build_cache:
        _build_cache[step] = build(step)
    nc = _build_cache[step]
    in_maps = host_inputs(hidden_states, Wq, bq, Wk, bk, Wv, bv, Wo, bo,
                          key_buffer, value_buffer)
    opts = _run_opts or {}
    r = run_bass_kernel_spmd(nc, in_maps, list(range(NCORES)), **opts)
    res = r.results
    out = np.stack([res[c]["out"] for c in range(NCORES)], 0).reshape(B, 1, D)
    kb = np.concatenate([res[c]["kb"] for c in range(NCORES)], 0)
    vb = np.concatenate([res[c]["vb"] for c in range(NCORES)], 0)
    if _run_opts is not None:
        return (out, kb, vb), r
    return out, kb, vb


# revision 16
# speedup vs baseline: 1.2092x; 1.2092x over previous
# MBartAttention single-step decode with KV-cache scatter, Trainium2 Bass/Tile.
#
# Sharding: data-parallel over batch. 16 batches / 8 cores = 2 batches per
# core; each core runs an identical program over its batch shard (no
# collectives). Host pre-transposes the (tiny) weight matrices so the device
# never has to transpose DRAM-resident weights.
#
# Per-core device work:
#   q/k/v = hs @ W.T + b       (PE; contraction over D on partitions; biases
#                               via an appended ones-row / bias-row)
#   kb/vb = scatter(step)      (head [0,step) streamed through SBUF and copied
#                               out; tail (step, S) copied DRAM->DRAM; row
#                               `step` written from the fresh k/v projection)
#   scores = K . q             (PE; K tiles transposed on PE via identity, then
#                               used as stationary so scores land with s on
#                               partitions -- the same layout V is consumed in)
#   softmax                    (ACT exp with per-(b,h) accumulate; partition
#                               reduction + reciprocal broadcast via tiny PE
#                               ones-matmuls; no max subtraction -- scores are
#                               O(1) here so exp cannot overflow)
#   ctx = sum_s p[s] V[s]      (PE; V consumed in natural [s, hd] layout from
#                               the same SBUF-resident copy used for copy-out)
#   out = ctx @ Wo.T + bo      (PE)
#
# The kernel is memory-bound: per core it must read K,V heads+tails+weights
# (~80 MiB) and write the updated caches (~64 MiB).

import numpy as np

import concourse.bass as bass  # noqa: F401
import concourse.mybir as mybir
import concourse.tile as tile
from concourse import bacc
from concourse.bass_utils import run_bass_kernel_spmd
from concourse.masks import make_identity

FP32 = mybir.dt.float32
AX = mybir.AxisListType
ALU = mybir.AluOpType
EXP = mybir.ActivationFunctionType.Exp

B, H, S, HD = 16, 16, 4096, 64
D = H * HD              # 1024
NCORES = 8
BL = B // NCORES        # 2 batches per core
NBH = BL * H            # 32 (batch, head) pairs per core


def _emit(tc, step, hsT, wqkvT, woT, bo_c, kbuf, vbuf, out_d, kb_d, vb_d):
    nc = tc.nc
    HEAD = step             # cached positions attended: 0..step-1 (+ new at step)
    assert HEAD % 512 == 0
    FPP = HEAD // 128       # head positions per SBUF partition (j slots)
    JP1 = FPP + 1           # + slot for the freshly projected position `step`
    NCH = HEAD // 512       # 512-column K-transpose chunks
    DT = D // 128           # din tiles
    DTP = DT + 1            # + bias (ones-row) tile

    with (
        tc.tile_pool(name="const", bufs=1) as cpool,
        tc.tile_pool(name="vres", bufs=1) as vpool,
        tc.tile_pool(name="small", bufs=1) as spool,
    ):
        ident = cpool.tile([128, 128], FP32, name="ident")
        make_identity(nc, ident[:])
        ones1 = cpool.tile([1, 1], FP32, name="ones1")
        nc.vector.memset(ones1[:], 1.0)
        ones_col = cpool.tile([128, 1], FP32, name="ones_col")
        nc.vector.memset(ones_col[:], 1.0)
        ones_row = cpool.tile([1, 128], FP32, name="ones_row")
        nc.vector.memset(ones_row[:], 1.0)
        bo_sb = cpool.tile([1, D], FP32, name="bo_sb")
        nc.sync.dma_start(bo_sb[:], bo_c[:, :])

        hsT_sb = spool.tile([128, DTP, BL], FP32, name="hsT_sb")
        nc.sync.dma_start(hsT_sb[:], hsT[:, :, :])

        # V head cache, resident: [s%128, s//128, bh, hd]; j slot FPP holds the
        # new position `step` in partition 0 (other partitions zeroed).
        v_sb = vpool.tile([128, JP1, NBH, HD], FP32, name="v_sb")
        nc.vector.memset(v_sb[:, FPP, :, :], 0.0)

        # exp(scores), layout [s%128, h, j, b]; j slot FPP = position `step`
        # (partition 0; rest zeroed).
        probs = spool.tile([128, H, JP1, BL], FP32, name="probs")
        nc.vector.memset(probs[:, :, FPP, :], 0.0)
        acc_sb = spool.tile([128, NBH], FP32, name="acc_sb")

        qT_sb = spool.tile([64, H, BL], FP32, name="qT_sb")
        knT_sb = spool.tile([64, H, BL], FP32, name="knT_sb")
        rd_row = spool.tile([1, NBH], FP32, name="rd_row")
        ctx_sb = spool.tile([64, NBH], FP32, name="ctx_sb")
        out_sb = spool.tile([1, BL, D], FP32, name="out_sb")

        # ---------------- q/k/v projections ----------------
        with (
            tc.tile_pool(name="wqkv", bufs=2) as wpool,
            tc.tile_pool(name="proj_ps", bufs=1, space="PSUM") as proj_psum,
            tc.tile_pool(name="tr_ps", bufs=1, space="PSUM") as tr_psum,
            tc.tile_pool(name="qkvp", bufs=1) as qkvp,
        ):
            qp_sb = qkvp.tile([BL, D], FP32, name="qp_sb")
            knp_sb = qkvp.tile([BL, D], FP32, name="knp_sb")
            vnp_sb = qkvp.tile([BL, D], FP32, name="vnp_sb")
            pps = [proj_psum.tile([BL, 512], FP32, name=f"pp{c}") for c in range(6)]
            for d in range(DTP):
                w_sb = wpool.tile([128, 3 * D], FP32, name="w_sb", tag="w")
                nc.sync.dma_start(w_sb[:], wqkvT[128 * d:128 * (d + 1), :])
                for c in range(6):
                    nc.tensor.matmul(
                        pps[c][:], hsT_sb[:, d, :], w_sb[:, 512 * c:512 * (c + 1)],
                        start=(d == 0), stop=(d == DTP - 1),
                    )
            for c in range(2):
                sl = slice(512 * c, 512 * (c + 1))
                nc.scalar.copy(qp_sb[:, sl], pps[c][:])
                nc.scalar.copy(knp_sb[:, sl], pps[2 + c][:])
                nc.scalar.copy(vnp_sb[:, sl], pps[4 + c][:])

            # scatter the new K/V row into the caches at position `step`
            nc.sync.dma_start(
                kb_d[:, :, step, :], knp_sb[:].rearrange("b (h e) -> b h e", h=H))
            nc.sync.dma_start(
                vb_d[:, :, step, :], vnp_sb[:].rearrange("b (h e) -> b h e", h=H))
            # and into the resident V buffer (slot FPP, partition 0)
            for b in range(BL):
                nc.sync.dma_start(
                    v_sb[0:1, FPP, H * b:H * (b + 1), :],
                    vnp_sb[b:b + 1, :].rearrange("p (h e) -> p h e", h=H))

            # q, k_new transposed to [hd, h, b] for PE score matmuls
            qT_ps = tr_psum.tile([64, H * BL], FP32, name="qT_ps")
            knT_ps = tr_psum.tile([64, H * BL], FP32, name="knT_ps")
            for h in range(H):
                sl = slice(64 * h, 64 * (h + 1))
                nc.tensor.transpose(
                    qT_ps[:, h * BL:(h + 1) * BL], qp_sb[:, sl], ident[:BL, :BL])
                nc.tensor.transpose(
                    knT_ps[:, h * BL:(h + 1) * BL], knp_sb[:, sl], ident[:BL, :BL])
            nc.vector.tensor_copy(qT_sb[:].rearrange("p h b -> p (h b)"), qT_ps[:])
            nc.vector.tensor_copy(knT_sb[:].rearrange("p h b -> p (h b)"), knT_ps[:])

        # ---------------- K/V stream + per-(b,h) attention pipeline ----------
        # Wo piece 0 is prefetched here so its DMA overlaps the K/V stream.
        with tc.tile_pool(name="wo", bufs=1) as wo_pool:
            HP = H // 2
            wop = wo_pool.tile([64, HP, D], FP32, name="wop", tag="wop")
            nc.sync.dma_start(
                wop[:], woT[0:64 * HP, :].rearrange("(h p) n -> p h n", p=64))

            with (
                tc.tile_pool(name="kst", bufs=2) as kpool,
                tc.tile_pool(name="ktp", bufs=2, space="PSUM") as kt_psum,
                tc.tile_pool(name="kts", bufs=2) as kts_pool,
                tc.tile_pool(name="sc_ps", bufs=2, space="PSUM") as sc_psum,
                tc.tile_pool(name="msc_ps", bufs=2, space="PSUM") as msc_psum,
                tc.tile_pool(name="ctx_ps", bufs=1, space="PSUM") as ctx_psum,
            ):
                ctxp = ctx_psum.tile([64, NBH], FP32, name="ctxp")
                for b in range(BL):
                    for h in range(H):
                        bh = b * H + h
                        k_sb = kpool.tile([128, FPP, HD], FP32, name="k_sb", tag="k")
                        nc.sync.dma_start(
                            k_sb[:],
                            kbuf[b, h, 0:HEAD, :].rearrange("(p f) e -> p f e", p=128))
                        nc.scalar.dma_start(
                            kb_d[b, h, 0:HEAD, :].rearrange("(p f) e -> p f e", p=128),
                            k_sb[:])
                        nc.scalar.dma_start(
                            kb_d[b, h, step + 1:S, :], kbuf[b, h, step + 1:S, :])
                        # V: load head into resident buffer, copy out, tail D2D
                        nc.sync.dma_start(
                            v_sb[:, 0:FPP, bh, :],
                            vbuf[b, h, 0:HEAD, :].rearrange("(p f) e -> p f e", p=128))
                        nc.scalar.dma_start(
                            vb_d[b, h, 0:HEAD, :].rearrange("(p f) e -> p f e", p=128),
                            v_sb[:, 0:FPP, bh, :])
                        nc.scalar.dma_start(
                            vb_d[b, h, step + 1:S, :], vbuf[b, h, step + 1:S, :])

                        # scores land [s%128, j] in a per-(b,h) PSUM bank
                        scb = sc_psum.tile([128, FPP], FP32, name="scb", tag="scb")
                        for c in range(NCH):
                            ktp = kt_psum.tile([64, 512], FP32, name="ktp", tag="ktp")
                            for j4 in range(4):
                                nc.tensor.transpose(
                                    ktp[:, 128 * j4:128 * (j4 + 1)],
                                    k_sb[:, 4 * c + j4, :], ident[:])
                            kts = kts_pool.tile([64, 512], FP32, name="kts", tag="kts")
                            if (bh + c) % 2 == 0:
                                nc.scalar.copy(kts[:], ktp[:])
                            else:
                                nc.vector.tensor_copy(kts[:], ktp[:])
                            for j4 in range(4):
                                j = 4 * c + j4
                                nc.tensor.matmul(
                                    scb[:, j:j + 1],
                                    kts[:, 128 * j4:128 * (j4 + 1)],
                                    qT_sb[:, h, b:b + 1], start=True, stop=True)
                        # msc columns: 0 = raw score at `step`, 1 = denom, 2 = 1/denom bcast
                        msc = msc_psum.tile([128, 4], FP32, name="msc", tag="msc")
                        nc.tensor.matmul(
                            msc[0:1, 0:1], knT_sb[:, h, b:b + 1], qT_sb[:, h, b:b + 1],
                            start=True, stop=True)

                        # exp (no max subtraction: scores here are O(1))
                        nc.scalar.activation(
                            probs[:, h, 0:FPP, b], scb[:, :], EXP,
                            accum_out=acc_sb[:, bh:bh + 1])
                        nc.scalar.activation(
                            probs[0:1, h, FPP:FPP + 1, b:b + 1], msc[0:1, 0:1], EXP)
                        # denom = sum over partitions + the `step` term
                        nc.tensor.matmul(
                            msc[0:1, 1:2], acc_sb[:, bh:bh + 1], ones_col[:],
                            start=True, stop=False)
                        nc.tensor.matmul(
                            msc[0:1, 1:2], probs[0:1, h, FPP, b:b + 1], ones1[:],
                            start=False, stop=True)
                        nc.vector.reciprocal(rd_row[0:1, bh:bh + 1], msc[0:1, 1:2])
                        nc.tensor.matmul(
                            msc[:, 2:3], ones_row[:], rd_row[0:1, bh:bh + 1],
                            start=True, stop=True)
                        nc.vector.tensor_scalar(
                            out=probs[:, h, :, b], in0=probs[:, h, :, b],
                            scalar1=msc[:, 2:3], scalar2=None, op0=ALU.mult)

                        # ctx column for this (b,h)
                        for m in range(JP1):
                            nc.tensor.matmul(
                                ctxp[:, bh:bh + 1], v_sb[:, m, bh, :],
                                probs[:, h, m, b:b + 1],
                                start=(m == 0), stop=(m == JP1 - 1))
                nc.vector.tensor_copy(ctx_sb[:], ctxp[:])

            # ---------------- out = ctx @ Wo.T + bo ----------------
            op_psum = tc.alloc_tile_pool(name="op_ps", bufs=1, space="PSUM")
            opst = [[op_psum.tile([1, 512], FP32, name=f"ops{c2}{b}")
                     for b in range(BL)] for c2 in range(2)]
            for c2 in range(2):
                for b in range(BL):
                    nc.tensor.matmul(
                        opst[c2][b][:], ones1[:],
                        bo_sb[0:1, 512 * c2:512 * (c2 + 1)],
                        start=True, stop=False)
            for pc in range(2):
                if pc == 1:
                    wop = wo_pool.tile([64, HP, D], FP32, name="wop", tag="wop")
                    nc.sync.dma_start(
                        wop[:],
                        woT[64 * HP:64 * HP * 2, :].rearrange("(h p) n -> p h n", p=64))
                for c2 in range(2):
                    for b in range(BL):
                        for hh in range(HP):
                            h = HP * pc + hh
                            nc.tensor.matmul(
                                opst[c2][b][:], ctx_sb[:, b * H + h:b * H + h + 1],
                                wop[:, hh, 512 * c2:512 * (c2 + 1)],
                                start=False, stop=(h == H - 1))
            for c2 in range(2):
                for b in range(BL):
                    nc.scalar.copy(
                        out_sb[0:1, b, 512 * c2:512 * (c2 + 1)], opst[c2][b][:])
            for b in range(BL):
                nc.sync.dma_start(out_d[b:b + 1, :], out_sb[0:1, b, :])
            op_psum.release()


def build(step):
    nc = bacc.Bacc("TRN2", target_bir_lowering=False, debug=False)
    hsT = nc.dram_tensor("hsT", [128, D // 128 + 1, BL], FP32, kind="ExternalInput")
    wqkvT = nc.dram_tensor("wqkvT", [D + 128, 3 * D], FP32, kind="ExternalInput")
    woT = nc.dram_tensor("woT", [D, D], FP32, kind="ExternalInput")
    bo_c = nc.dram_tensor("bo_c", [1, D], FP32, kind="ExternalInput")
    kbuf = nc.dram_tensor("kbuf", [BL, H, S, HD], FP32, kind="ExternalInput")
    vbuf = nc.dram_tensor("vbuf", [BL, H, S, HD], FP32, kind="ExternalInput")
    out_d = nc.dram_tensor("out", [BL, D], FP32, kind="ExternalOutput")
    kb_d = nc.dram_tensor("kb", [BL, H, S, HD], FP32, kind="ExternalOutput")
    vb_d = nc.dram_tensor("vb", [BL, H, S, HD], FP32, kind="ExternalOutput")
    with tile.TileContext(nc) as tc:
        _emit(tc, step, hsT, wqkvT, woT, bo_c, kbuf, vbuf, out_d, kb_d, vb_d)
    nc.compile()
    return nc


def host_inputs(hidden_states, Wq, bq, Wk, bk, Wv, bv, Wo, bo,
                key_buffer, value_buffer):
    """Build the per-core input maps (host-side layout prep only)."""
    hs = np.asarray(hidden_states, dtype=np.float32)
    scale = np.float32(HD) ** np.float32(-0.5)
    wq = np.asarray(Wq, np.float32) * scale
    bqs = np.asarray(bq, np.float32) * scale
    wqkvT = np.zeros((D + 128, 3 * D), np.float32)
    wqkvT[:D, 0 * D:1 * D] = wq.T
    wqkvT[:D, 1 * D:2 * D] = np.asarray(Wk, np.float32).T
    wqkvT[:D, 2 * D:3 * D] = np.asarray(Wv, np.float32).T
    wqkvT[D, 0 * D:1 * D] = bqs
    wqkvT[D, 1 * D:2 * D] = np.asarray(bk, np.float32)
    wqkvT[D, 2 * D:3 * D] = np.asarray(bv, np.float32)
    woT = np.ascontiguousarray(np.asarray(Wo, np.float32).T)
    bo_c = np.asarray(bo, np.float32).reshape(1, D)

    kb = np.asarray(key_buffer, np.float32)
    vb = np.asarray(value_buffer, np.float32)

    in_maps = []
    for c in range(NCORES):
        b0 = c * BL
        hsc = hs[b0:b0 + BL, 0, :]                       # [BL, D]
        hsT = np.zeros((128, D // 128 + 1, BL), np.float32)
        hsT[:, :D // 128, :] = hsc.T.reshape(D // 128, 128, BL).transpose(1, 0, 2)
        hsT[0, D // 128, :] = 1.0                        # ones row -> bias row
        in_maps.append({
            "hsT": hsT,
            "wqkvT": wqkvT,
            "woT": woT,
            "bo_c": bo_c,
            "kbuf": np.ascontiguousarray(kb[b0:b0 + BL]),
            "vbuf": np.ascontiguousarray(vb[b0:b0 + BL]),
        })
    return in_maps


_build_cache = {}


def kernel(hidden_states, key_buffer, value_buffer, Wq, bq, Wk, bk, Wv, bv,
           Wo, bo, step, _run_opts=None):
    step = int(step)
    if step not in _build_cache:
        _build_cache[step] = build(step)
    nc = _build_cache[step]
    in_maps = host_inputs(hidden_states, Wq, bq, Wk, bk, Wv, bv, Wo, bo,
                          key_buffer, value_buffer)
    opts = _run_opts or {}
    r = run_bass_kernel_spmd(nc, in_maps, list(range(NCORES)), **opts)
    res = r.results
    out = np.stack([res[c]["out"] for c in range(NCORES)], 0).reshape(B, 1, D)
    kb = np.concatenate([res[c]["kb"] for c in range(NCORES)], 0)
    vb = np.concatenate([res[c]["vb"] for c in range(NCORES)], 0)
    if _run_opts is not None:
        return (out, kb, vb), r
    return out, kb, vb
